# revision 89
# baseline (speedup 1.0000x reference)
"""HaciCognitiveNet Trainium2 kernel.

Data-parallel over batch: B=8 -> one batch element per NeuronCore.
Activations live TRANSPOSED on-chip ([D, S], D on partitions) in bf16.
All matmuls are bf16 with fp32 PSUM accumulation (full PE rate, no
fp32-HIGH power throttle, FWL halves LDWEIGHTS time).

LayerNorm over D (partition dim in this layout) is done with:
  - ones-column matmuls for sums / sum-of-squares -> [1,S] rows in PSUM
  - var via 3 DVE ops, sqrt(var+eps) on ACT, reciprocal_approx_fast (DVE)
  - mean correction folded into projections as rank-1 K=1 matmuls
  - rstd applied as per-column broadcast planes (K=1 broadcast matmuls)

The decay mask 0.99^(q-k) is separable: folded into per-column scales of
Q^T (0.99^q / 8) and K^T (0.99^-k). Only the 128x128 diagonal blocks
need triangular zeroing (one shared triu tile, DVE mul); off-diagonal
score regions drain as plain ACT copies, splitting drain load across
engines. The k_t=3 score block is narrowed to its valid q range.
Score matmuls run as row-tiled head pairs and AV matmuls as col-tiled
head pairs into one [128,512] PSUM bank (halves the ret drain count).
Inner-LN stats accumulate per head-pair during the scores phase. V's
rstd is folded into K's column scale (rstd^2), so the V drain is a
plain copy. W1/W2 are SBUF-resident; retention weights prefetch one
layer ahead.
"""

import numpy as np
import ml_dtypes

BF16 = ml_dtypes.bfloat16

B, S, DIN, D, H, FF = 8, 512, 384, 512, 8, 2048
DH = D // H
N_WM, N_CORE = 2, 4
NL = N_WM + N_CORE
DECAY = 0.99
EPS = 1e-5
PT = D // 128   # 4 partition tiles of the model dim
CT = S // 128   # 4 tiles of the sequence dim

_CACHE = {}


def _lhsT_layout(w):
    """[K, M] weight -> SBUF lhsT tile layout [128, (K//128)*M]."""
    k, m = w.shape
    c = k // 128
    return np.ascontiguousarray(
        w.reshape(c, 128, m).transpose(1, 0, 2).reshape(128, c * m)
    ).astype(np.float32)


def _build_program():
    import concourse.bass as bass
    import concourse.tile as tile
    from concourse import mybir, bacc
    from contextlib import ExitStack

    f32 = mybir.dt.float32
    bf16 = mybir.dt.bfloat16
    AF = mybir.ActivationFunctionType
    ALU = mybir.AluOpType

    nc = bacc.Bacc("TRN2", target_bir_lowering=False, debug=False)

    XT = nc.dram_tensor("xt", [128, 3, 512], bf16, kind="ExternalInput").ap()
    INW = nc.dram_tensor("inw", [128, 3 * 512], bf16, kind="ExternalInput").ap()
    INB = nc.dram_tensor("inb", [128, 4], f32, kind="ExternalInput").ap()
    WST = nc.dram_tensor("wst", [NL, 5, 128, 2048], bf16, kind="ExternalInput").ap()
    UST = nc.dram_tensor("ust", [NL, 1, 4 * 512], bf16, kind="ExternalInput").ap()
    BG = nc.dram_tensor("bg", [NL, 128, 4], f32, kind="ExternalInput").ap()
    BO = nc.dram_tensor("bo", [NL, 128, 4], f32, kind="ExternalInput").ap()
    W1T = nc.dram_tensor("w1t", [128, 4, 2048], bf16, kind="ExternalInput").ap()
    W2T = nc.dram_tensor("w2t", [128, 16 * 512], bf16, kind="ExternalInput").ap()
    B1C = nc.dram_tensor("b1c", [128, 16], f32, kind="ExternalInput").ap()
    B2C = nc.dram_tensor("b2c", [128, 4], f32, kind="ExternalInput").ap()
    ONWC = nc.dram_tensor("onwc", [128, 4], f32, kind="ExternalInput").ap()
    ONBC = nc.dram_tensor("onbc", [128, 4], f32, kind="ExternalInput").ap()
    CQ = nc.dram_tensor("cq", [1, 512], f32, kind="ExternalInput").ap()
    DK = nc.dram_tensor("dk", [1, 512], f32, kind="ExternalInput").ap()
    TRIU = nc.dram_tensor("triu", [128, 128], f32, kind="ExternalInput").ap()
    ONESC = nc.dram_tensor("onesc", [128, 1], bf16, kind="ExternalInput").ap()
    ONESR = nc.dram_tensor("onesr", [1, 128], bf16, kind="ExternalInput").ap()
    HOUT = nc.dram_tensor("hout", [4, 128, 512], bf16, kind="ExternalOutput").ap()

    with tile.TileContext(nc) as tc:
        with ExitStack() as ctx:
            consts = ctx.enter_context(tc.tile_pool(name="consts", bufs=1))
            wpool = ctx.enter_context(tc.tile_pool(name="wpool", bufs=10))
            wsmall = ctx.enter_context(tc.tile_pool(name="wsmall", bufs=2))
            hpool = ctx.enter_context(tc.tile_pool(name="hpool", bufs=2))
            apool = ctx.enter_context(tc.tile_pool(name="apool", bufs=1))
            atpool = ctx.enter_context(tc.tile_pool(name="atpool", bufs=16))
            spool = ctx.enter_context(tc.tile_pool(name="spool", bufs=6))
            sqpool = ctx.enter_context(tc.tile_pool(name="sqpool", bufs=4))
            grpool = ctx.enter_context(tc.tile_pool(name="grpool", bufs=4))
            retpool = ctx.enter_context(tc.tile_pool(name="retpool", bufs=1))
            plpool = ctx.enter_context(tc.tile_pool(name="plpool", bufs=3))
            odpool = ctx.enter_context(tc.tile_pool(name="odpool", bufs=2))
            f1pool = ctx.enter_context(tc.tile_pool(name="f1pool", bufs=3))
            psum = ctx.enter_context(tc.tile_pool(name="psum", bufs=5, space="PSUM"))
            bcps = ctx.enter_context(tc.tile_pool(name="bcps", bufs=2, space="PSUM"))
            rpsum = ctx.enter_context(tc.tile_pool(name="rpsum", bufs=1, space="PSUM"))

            # ---- consts ----
            cq_sb = consts.tile([1, 512], f32)
            nc.sync.dma_start(out=cq_sb[:], in_=CQ[:])
            dk_sb = consts.tile([1, 512], f32)
            nc.sync.dma_start(out=dk_sb[:], in_=DK[:])
            triu_sb = consts.tile([128, 128], f32)
            nc.sync.dma_start(out=triu_sb[:], in_=TRIU[:])
            onesc_sb = consts.tile([128, 1], bf16)
            nc.sync.dma_start(out=onesc_sb[:], in_=ONESC[:])
            onesr_sb = consts.tile([1, 128], bf16)
            nc.sync.dma_start(out=onesr_sb[:], in_=ONESR[:])
            onw_sb = consts.tile([128, 4], f32)
            nc.sync.dma_start(out=onw_sb[:], in_=ONWC[:])
            onb_sb = consts.tile([128, 4], f32)
            nc.sync.dma_start(out=onb_sb[:], in_=ONBC[:])
            b1_sb = consts.tile([128, 16], f32)
            nc.sync.dma_start(out=b1_sb[:], in_=B1C[:])
            b2_sb = consts.tile([128, 4], f32)
            nc.sync.dma_start(out=b2_sb[:], in_=B2C[:])
            eps_sb = consts.tile([128, 1], f32)
            nc.gpsimd.memset(eps_sb[:], EPS)
            # resident FFN weights (bf16: 2 MB each)
            w1_sb = consts.tile([128, 4, 2048], bf16)
            nc.sync.dma_start(out=w1_sb[:], in_=W1T[:])
            w2_sb = consts.tile([128, 16 * 512], bf16)
            nc.sync.dma_start(out=w2_sb[:], in_=W2T[:])

            # ---- PE warm-up burst ----
            # The PE is idle for ~10us at kernel start while input/weight
            # DMAs stream. ~5us of back-to-back dummy matmuls here crosses
            # the HAM activity window, so the clock gate opens to 8/8
            # before the first real matmul regardless of the throttle
            # state the device was left in (this start-state is the likely
            # cause of occasional ~20% slower whole-run timings).
            wa = consts.tile([128, 128], bf16)
            nc.gpsimd.memset(wa[:], 0.0)
            wm = consts.tile([128, 512], bf16)
            nc.gpsimd.memset(wm[:], 0.0)
            wp = psum.tile([128, 512], f32, tag="big")
            for _ in range(24):
                nc.tensor.matmul(wp[:], wa[:], wm[:], start=True, stop=True)

            # ---- input projection: ht = (x @ in_w + in_b)^T ----
            inctx = ExitStack()
            inpool = inctx.enter_context(tc.tile_pool(name="inpool", bufs=1))
            xt_sb = inpool.tile([128, 3, 512], bf16)
            nc.sync.dma_start(out=xt_sb[:], in_=XT[:])
            inw_sb = inpool.tile([128, 3 * 512], bf16)
            nc.sync.dma_start(out=inw_sb[:], in_=INW[:])
            inb_sb = inpool.tile([128, 4], f32)
            nc.sync.dma_start(out=inb_sb[:], in_=INB[:])
            ht = []
            for j in range(PT):
                p = psum.tile([128, 512], f32, tag="big")
                for c in range(3):
                    nc.tensor.matmul(
                        p[:], inw_sb[:, c * 512 + 128 * j : c * 512 + 128 * (j + 1)],
                        xt_sb[:, c, :], start=(c == 0), stop=(c == 2))
                hj = hpool.tile([128, 512], bf16, tag=f"ht{j}")
                nc.scalar.activation(hj[:], p[:], AF.Identity, bias=inb_sb[:, j : j + 1])
                ht.append(hj)
            inctx.close()

            def load_weights(lidx):
                wmat = []
                for i in range(5):
                    wt = wpool.tile([128, 2048], bf16, tag="wmat")
                    nc.sync.dma_start(out=wt[:], in_=WST[lidx, i])
                    wmat.append(wt)
                u_sb = wsmall.tile([1, 4 * 512], bf16, tag="urow")
                nc.sync.dma_start(out=u_sb[:], in_=UST[lidx])
                bg_sb = wsmall.tile([128, 4], f32, tag="bgc")
                nc.sync.dma_start(out=bg_sb[:], in_=BG[lidx])
                bo_sb = wsmall.tile([128, 4], f32, tag="boc")
                nc.sync.dma_start(out=bo_sb[:], in_=BO[lidx])
                return wmat, u_sb, bg_sb, bo_sb

            def layer_stats(tiles, sq_tiles):
                """ones-matmul stats -> (sums_ps, ssq_ps) [1,512] PSUM tiles."""
                sums = bcps.tile([1, 512], f32, tag="bc")
                ssq = bcps.tile([1, 512], f32, tag="bc")
                for j in range(PT):
                    nc.tensor.matmul(sums[:], onesc_sb[:], tiles[j][:],
                                     start=(j == 0), stop=(j == PT - 1))
                for j in range(PT):
                    nc.tensor.matmul(ssq[:], onesc_sb[:], sq_tiles[j][:],
                                     start=(j == 0), stop=(j == PT - 1))
                return sums, ssq

            def rsqrt_row(vrow_ps, ssq_ps, pool):
                """sums/ssq psum rows -> (negmu bf16, rb bf16 [1,512]).

                var via 3 DVE ops, sqrt(var+eps) on ACT (~570ns, short
                serial latency), 1/std via the single-instruction DVE
                reciprocal_approx_fast."""
                negmu = pool.tile([1, 512], bf16, tag="tiny")
                nc.vector.tensor_scalar_mul(negmu[:], vrow_ps[:], -1.0 / D)
                nm2 = pool.tile([1, 512], f32, tag="tiny")
                nc.vector.tensor_mul(nm2[:], negmu[:], negmu[:])
                w = pool.tile([1, 512], f32, tag="tiny")
                nc.vector.scalar_tensor_tensor(w[:], ssq_ps[:], 1.0 / D, nm2[:],
                                               ALU.mult, ALU.subtract)
                std = pool.tile([1, 512], f32, tag="tiny")
                nc.scalar.activation(std[:], w[:], AF.Sqrt, bias=eps_sb[0:1, :])
                r = pool.tile([1, 512], f32, tag="tiny")
                nc.vector.reciprocal_approx_fast(r[:], std[:])
                rb = pool.tile([1, 512], bf16, tag="tiny")
                nc.vector.tensor_scalar_mul(rb[:], r[:], 1.0)
                return negmu, r, rb

            def bcast_plane(row_bf16):
                """[1,512] bf16 row -> [128,512] bf16 SBUF plane."""
                p = bcps.tile([128, 512], f32, tag="bc")
                nc.tensor.matmul(p[:], onesr_sb[:], row_bf16[:], start=True, stop=True)
                sb = plpool.tile([128, 512], bf16, tag="plane")
                nc.scalar.copy(sb[:], p[:])
                return sb

            def retention(lidx, weights, next_weights_hook=None):
                wmat, u_sb, bg_sb, bo_sb = weights

                # V main matmul groups FIRST: they depend only on ht, so
                # the PE chews them while the ACT Square -> stats chain
                # runs, instead of stalling on the stats matmuls at the
                # layer boundary (pipeline skew #4). V's rank-1 correction
                # follows the stats below.
                pV = []
                for j in range(PT):
                    p = psum.tile([128, 512], f32, tag="big")
                    for c in range(PT):
                        nc.tensor.matmul(
                            p[:], ht[c][:, 128 * j : 128 * (j + 1)],
                            wmat[2][:, c * 512 : (c + 1) * 512],
                            start=(c == 0), stop=False)
                    pV.append(p)

                # pre-LN stats
                sq = []
                for j in range(PT):
                    s = sqpool.tile([128, 512], bf16, tag="sq")
                    nc.scalar.activation(s[:], ht[j][:], AF.Square)
                    sq.append(s)
                sums, ssq = layer_stats(ht, sq)
                negmu, r, rb = rsqrt_row(sums, ssq, spool)
                qs = spool.tile([1, 512], bf16, tag="tiny")
                nc.vector.tensor_mul(qs[:], r[:], cq_sb[:])
                r2 = spool.tile([1, 512], f32, tag="tiny")
                nc.vector.tensor_mul(r2[:], r[:], r[:])
                ks = spool.tile([1, 512], bf16, tag="tiny")
                nc.vector.tensor_mul(ks[:], r2[:], dk_sb[:])
                qs_b = ks_b = g_b = None

                # prefetch next layer's weights while this layer computes
                nxt = next_weights_hook() if next_weights_hook else None

                # V rank-1 corrections (negmu is one DVE op past the stats)
                # and dependency-free V drains
                vn = []
                for j in range(PT):
                    nc.tensor.matmul(
                        pV[j][:], negmu[:, 128 * j : 128 * (j + 1)],
                        u_sb[:, 2 * 512 : 2 * 512 + 512],
                        start=False, stop=True)
                for j in range(PT):
                    t = apool.tile([128, 512], bf16, tag=f"proj2_{j}")
                    nc.scalar.copy(t[:], pV[j][:])
                    vn.append(t)

                qt, kt, gt = [], [], []
                for ip, dest in ((0, qt), (1, kt), (3, gt)):
                    pss = []
                    for j in range(PT):
                        p = psum.tile([128, 512], f32, tag="big")
                        for c in range(PT):
                            nc.tensor.matmul(
                                p[:],
                                wmat[ip][:, c * 512 + 128 * j : c * 512 + 128 * (j + 1)],
                                ht[c][:], start=(c == 0), stop=False)
                        pss.append(p)
                    for j in range(PT):
                        nc.tensor.matmul(
                            pss[j][:],
                            u_sb[:, ip * 512 + 128 * j : ip * 512 + 128 * (j + 1)],
                            negmu[:], start=False, stop=True)
                    if ip == 0:
                        qs_b = bcast_plane(qs)
                        ks_b = bcast_plane(ks)
                        g_b = bcast_plane(rb)
                    for j in range(PT):
                        p = pss[j]
                        t = apool.tile([128, 512], bf16, tag=f"proj{ip}_{j}")
                        if ip == 0:
                            nc.vector.tensor_mul(t[:], p[:], qs_b[:])
                        elif ip == 1:
                            nc.vector.tensor_mul(t[:], p[:], ks_b[:])
                        else:
                            tg = odpool.tile([128, 512], bf16, tag="gtmp")
                            nc.vector.tensor_mul(tg[:], p[:], g_b[:])
                            nc.scalar.activation(t[:], tg[:], AF.Sigmoid,
                                                 bias=bg_sb[:, j : j + 1])
                        dest.append(t)

                # scores + AV per head pair (heads 2jt, 2jt+1 share q/k tile
                # jt); inner-LN stats accumulate per jt as ret tiles land, so
                # only the rsqrt chain remains after the last AV drain
                ret_sb = []
                sums2 = bcps.tile([1, 512], f32, tag="bc")
                ssq2 = bcps.tile([1, 512], f32, tag="bc")

                def score_block(jt):
                    # score matmuls interleaved per k_t: head A's lhsT at
                    # rows 0-63, head B's at 64-127 -> concurrent (row tiling)
                    at_pair = [[], []]
                    for k_t in range(CT):
                        qstart = 128 * k_t
                        npr = 512 - qstart
                        scp = []
                        for i, r0 in enumerate((0, 64)):
                            sc = psum.tile([128, 512], f32, tag="big")
                            nc.tensor.matmul(
                                sc[:, 0:npr],
                                kt[jt][r0 : r0 + 64, 128 * k_t : 128 * (k_t + 1)],
                                qt[jt][r0 : r0 + 64, qstart : 512],
                                start=True, stop=True)
                            scp.append(sc)
                        for i in range(2):
                            sc = scp[i]
                            at = atpool.tile([128, 512], bf16, tag="at")
                            # diagonal 128-block: triangular mask (DVE)
                            nc.vector.tensor_mul(
                                at[:, qstart : qstart + 128],
                                sc[:, 0:128], triu_sb[:])
                            # off-diagonal region: plain drain (ACT)
                            if npr > 128:
                                nc.scalar.copy(
                                    at[:, qstart + 128 : 512], sc[:, 128:npr])
                            at_pair[i].append(at)
                    return at_pair

                # pipeline skew of one head pair: the score matmuls for
                # jt+1 are issued BEFORE the AV matmuls for jt, so the
                # in-order PE runs them while jt's at drains complete
                # instead of stalling in front of AV. The inner-LN stats
                # matmuls are deferred one iteration for the same reason:
                # they wait on the ACT ret-copy+Square chain and would
                # otherwise gate the next score block in the PE queue.
                def emit_stats(pend):
                    prs, ps2, pjt = pend
                    nc.tensor.matmul(sums2[:], onesc_sb[:], prs[:],
                                     start=(pjt == 0), stop=(pjt == PT - 1))
                    nc.tensor.matmul(ssq2[:], onesc_sb[:], ps2[:],
                                     start=(pjt == 0), stop=(pjt == PT - 1))

                cur = score_block(0)
                pend = None
                for jt in range(PT):
                    nxt_at = score_block(jt + 1) if jt + 1 < PT else None
                    if pend is not None:
                        emit_stats(pend)
                    at_pair = cur
                    # AV pair interleaved per k_t: head A -> PSUM cols 0-63,
                    # head B -> 64-127 (col tiling, concurrent)
                    rp = rpsum.tile([128, 512], f32, tag="ret")
                    for k_t in range(CT):
                        qstart = 128 * k_t
                        for i in range(2):
                            h = 2 * jt + i
                            nc.tensor.matmul(
                                rp[64 * i : 64 * i + 64, qstart : 512],
                                vn[k_t][:, 64 * h : 64 * (h + 1)],
                                at_pair[i][k_t][:, qstart : 512],
                                start=(k_t == 0), stop=(k_t == CT - 1),
                                tile_position=(0, 64 * i))
                    rs = retpool.tile([128, 512], bf16, tag=f"ret{jt}")
                    nc.scalar.copy(rs[:], rp[:])
                    ret_sb.append(rs)
                    s2 = sqpool.tile([128, 512], bf16, tag="sq")
                    nc.scalar.activation(s2[:], rs[:], AF.Square)
                    pend = (rs, s2, jt)
                    cur = nxt_at
                emit_stats(pend)

                negmu2, rB, rBb = rsqrt_row(sums2, ssq2, spool)
                nm2_b = bcast_plane(negmu2)

                # gret = (ret - mu2) * g; rstd2 is applied at the O drain so
                # the O matmuls only wait on negmu2, not the whole chain
                gret = []
                for j in range(PT):
                    tmpc = odpool.tile([128, 512], bf16, tag="odb")
                    nc.vector.tensor_add(tmpc[:], ret_sb[j][:], nm2_b[:])
                    gr = grpool.tile([128, 512], bf16, tag="gret")
                    nc.vector.tensor_mul(gr[:], tmpc[:], gt[j][:])
                    gret.append(gr)

                # O = rstd2*T1 + bo + h, matmuls c-outer to start on gret[0]
                p1s = []
                for j in range(PT):
                    p1 = psum.tile([128, 512], f32, tag="big")
                    p1s.append(p1)
                for c in range(PT):
                    for j in range(PT):
                        nc.tensor.matmul(
                            p1s[j][:],
                            wmat[4][:, c * 512 + 128 * j : c * 512 + 128 * (j + 1)],
                            gret[c][:], start=(c == 0), stop=(c == PT - 1))
                    if c == 0:
                        rstd2_b = bcast_plane(rBb)
                newht = []
                for j in range(PT):
                    a = odpool.tile([128, 512], bf16, tag="oda")
                    nc.vector.tensor_mul(a[:], p1s[j][:], rstd2_b[:])
                    hn = hpool.tile([128, 512], bf16, tag=f"ht{j}")
                    nc.vector.scalar_tensor_tensor(hn[:], a[:], bo_sb[:, j : j + 1],
                                                   ht[j][:], ALU.add, ALU.add)
                    newht.append(hn)
                for j in range(PT):
                    ht[j] = newht[j]
                return nxt

            def ffn():
                f2ps = []
                for _j in range(PT):
                    f2p = psum.tile([128, 512], f32, tag="big")
                    f2ps.append(f2p)

                def w1_group(t):
                    # W1 PSUM tiles come from the bcps pool (idle during the
                    # FFN) so two can be in flight alongside the 4 f2ps banks
                    p = bcps.tile([128, 512], f32, tag="bc")
                    for c in range(PT):
                        nc.tensor.matmul(
                            p[:], w1_sb[:, c, 128 * t : 128 * (t + 1)], ht[c][:],
                            start=(c == 0), stop=(c == PT - 1))
                    return p

                # software-pipeline skew of one tile: W1 for t+1 is issued
                # BEFORE W2 for t, so the in-order PE never stalls on the
                # gelu drain between a W1 group and its W2 group
                pW = w1_group(0)
                for t in range(16):
                    pN = w1_group(t + 1) if t + 1 < 16 else None
                    f1 = f1pool.tile([128, 512], bf16, tag="f1")
                    nc.scalar.activation(f1[:], pW[:], AF.Gelu, bias=b1_sb[:, t : t + 1])
                    for j in range(PT):
                        nc.tensor.matmul(
                            f2ps[j][:], w2_sb[:, t * 512 + 128 * j : t * 512 + 128 * (j + 1)],
                            f1[:], start=(t == 0), stop=(t == 15))
                    pW = pN
                for j in range(PT):
                    hn = hpool.tile([128, 512], bf16, tag=f"ht{j}")
                    nc.vector.scalar_tensor_tensor(hn[:], f2ps[j][:], b2_sb[:, j : j + 1],
                                                   ht[j][:], ALU.add, ALU.add)
                    ht[j] = hn

            # layer schedule with one-layer weight prefetch
            weights = load_weights(0)
            for l in range(N_WM):
                hook = (lambda nl=l + 1: load_weights(nl)) if l + 1 < NL else None
                weights = retention(l, weights, hook) or weights
                ffn()

            # final LN of world model
            sqf = []
            for j in range(PT):
                s = sqpool.tile([128, 512], bf16, tag="sq")
                nc.scalar.activation(s[:], ht[j][:], AF.Square)
                sqf.append(s)
            sumsf, ssqf = layer_stats(ht, sqf)
            negmuf, rf, rfb = rsqrt_row(sumsf, ssqf, spool)
            nmr = spool.tile([1, 512], bf16, tag="tiny")
            nc.vector.tensor_mul(nmr[:], negmuf[:], rf[:])
            rf_b = bcast_plane(rfb)
            nmr_b = bcast_plane(nmr)
            for j in range(PT):
                t1 = odpool.tile([128, 512], bf16, tag="oda")
                nc.vector.tensor_mul(t1[:], ht[j][:], rf_b[:])
                t2 = odpool.tile([128, 512], bf16, tag="odb")
                nc.vector.tensor_add(t2[:], t1[:], nmr_b[:])
                hn = hpool.tile([128, 512], bf16, tag=f"ht{j}")
                nc.vector.tensor_scalar(hn[:], t2[:], onw_sb[:, j : j + 1],
                                        onb_sb[:, j : j + 1], ALU.mult, ALU.add)
                ht[j] = hn

            # retention core layers
            for l in range(N_WM, NL):
                hook = (lambda nl=l + 1: load_weights(nl)) if l + 1 < NL else None
                weights = retention(l, weights, hook) or weights

            for j in range(PT):
                nc.sync.dma_start(out=HOUT[j], in_=ht[j][:])

    nc.compile()
    return nc


def _host_prep(inputs):
    """Fold weights host-side; returns the shared in_map dict (no xt)."""
    g = {k: np.asarray(v, dtype=np.float32) for k, v in inputs.items()}

    def layer_params(l):
        if l < N_WM:
            pre = "wm_"
            i = l
        else:
            pre = "co_"
            i = l - N_WM
        return {n: g[pre + n][i] for n in
                ("wq", "bq", "wk", "bk", "wv", "bv", "wg", "bg", "wo", "bo",
                 "lnw", "lnb", "prew", "preb")}

    wst = np.zeros((NL, 5, 128, 2048), np.float32)
    ust = np.zeros((NL, 1, 4 * 512), np.float32)
    bgc = np.zeros((NL, 128, 4), np.float32)
    boc = np.zeros((NL, 128, 4), np.float32)
    for l in range(NL):
        p = layer_params(l)
        wq = p["prew"][:, None] * p["wq"]
        wk = p["prew"][:, None] * p["wk"]
        wv = p["prew"][:, None] * p["wv"]
        wg = p["prew"][:, None] * p["wg"]
        wo = p["lnw"][:, None] * p["wo"]
        # biases bq~ = bq + preb @ wq must be zero for this folded fast path
        for nm, w in (("bq", p["wq"]), ("bk", p["wk"]), ("bv", p["wv"])):
            bb = p[nm] + p["preb"] @ w
            assert np.abs(bb).max() == 0.0, f"nonzero {nm} not supported"
        assert np.abs(p["lnb"]).max() == 0.0, "nonzero lnb not supported"
        bgf = p["bg"] + p["preb"] @ p["wg"]
        wst[l, 0] = _lhsT_layout(wq)
        wst[l, 1] = _lhsT_layout(wk)
        wst[l, 2] = _lhsT_layout(wv)
        wst[l, 3] = _lhsT_layout(wg)
        wst[l, 4] = _lhsT_layout(wo)
        ust[l, 0, 0:512] = wq.sum(0)
        ust[l, 0, 512:1024] = wk.sum(0)
        ust[l, 0, 1024:1536] = wv.sum(0)
        ust[l, 0, 1536:2048] = wg.sum(0)
        bgc[l] = bgf.reshape(4, 128).T
        boc[l] = p["bo"].reshape(4, 128).T

    inw = _lhsT_layout(g["in_w"])
    inb = g["in_b"].reshape(4, 128).T.copy()
    w1t = _lhsT_layout(g["ffn_w1"]).reshape(128, 4, 2048)
    w2t = _lhsT_layout(g["ffn_w2"])  # [128, 16*512]
    b1c = g["ffn_b1"].reshape(16, 128).T.copy()
    b2c = g["ffn_b2"].reshape(4, 128).T.copy()
    onwc = g["wm_onw"].reshape(4, 128).T.copy()
    onbc = g["wm_onb"].reshape(4, 128).T.copy()

    q = np.arange(S, dtype=np.float64)
    cq = (DECAY ** q / np.sqrt(DH)).astype(np.float32).reshape(1, 512)
    dk = (DECAY ** (-q)).astype(np.float32).reshape(1, 512)
    triu = np.triu(np.ones((128, 128), np.float32))

    return {
        "inw": inw.astype(BF16), "inb": inb,
        "wst": wst.astype(BF16), "ust": ust.astype(BF16),
        "bg": bgc, "bo": boc,
        "w1t": np.ascontiguousarray(w1t).astype(BF16),
        "w2t": w2t.astype(BF16), "b1c": b1c, "b2c": b2c,
        "onwc": onwc, "onbc": onbc, "cq": cq, "dk": dk, "triu": triu,
        "onesc": np.ones((128, 1), BF16),
        "onesr": np.ones((1, 128), BF16),
    }


def _make_in_maps(inputs):
    shared = _host_prep(inputs)
    x = np.asarray(inputs["x"], dtype=np.float32)
    in_maps = []
    for b in range(B):
        xt = np.ascontiguousarray(
            x[b].T.reshape(3, 128, 512).transpose(1, 0, 2)).astype(BF16)
        m = dict(shared)
        m["xt"] = xt
        in_maps.append(m)
    return in_maps


def kernel(**inputs):
    from concourse.bass_utils import run_bass_kernel_spmd

    if "nc" not in _CACHE:
        _CACHE["nc"] = _build_program()
    nc = _CACHE["nc"]

    in_maps = _make_in_maps(inputs)
    res = run_bass_kernel_spmd(nc, in_maps, list(range(B)))
    out = np.empty((B, S, D), np.float32)
    for b in range(B):
        hout = res.results[b]["hout"]  # [4,128,512] = ht tiles (transposed h)
        out[b] = np.asarray(hout, dtype=np.float32).reshape(512, 512).T
    return out


# revision 90
# speedup vs baseline: 1.0117x; 1.0117x over previous
"""HaciCognitiveNet Trainium2 kernel.

Data-parallel over batch: B=8 -> one batch element per NeuronCore.
Activations live TRANSPOSED on-chip ([D, S], D on partitions) in bf16.
All matmuls are bf16 with fp32 PSUM accumulation (full PE rate, no
fp32-HIGH power throttle, FWL halves LDWEIGHTS time).

LayerNorm over D (partition dim in this layout) is done with:
  - ones-column matmuls for sums / sum-of-squares -> [1,S] rows in PSUM
  - var via 3 DVE ops, sqrt(var+eps) on ACT, reciprocal_approx_fast (DVE)
  - mean correction folded into projections as rank-1 K=1 matmuls
  - rstd applied as per-column broadcast planes (K=1 broadcast matmuls)

The decay mask 0.99^(q-k) is separable: folded into per-column scales of
Q^T (0.99^q / 8) and K^T (0.99^-k). Only the 128x128 diagonal blocks
need triangular zeroing (one shared triu tile, DVE mul); off-diagonal
score regions drain as plain ACT copies, splitting drain load across
engines. The k_t=3 score block is narrowed to its valid q range.
Score matmuls run as row-tiled head pairs and AV matmuls as col-tiled
head pairs into one [128,512] PSUM bank (halves the ret drain count).
Inner-LN stats accumulate per head-pair during the scores phase. V's
rstd is folded into K's column scale (rstd^2), so the V drain is a
plain copy. W1/W2 are SBUF-resident; retention weights prefetch one
layer ahead.
"""

import numpy as np
import ml_dtypes

BF16 = ml_dtypes.bfloat16

B, S, DIN, D, H, FF = 8, 512, 384, 512, 8, 2048
DH = D // H
N_WM, N_CORE = 2, 4
NL = N_WM + N_CORE
DECAY = 0.99
EPS = 1e-5
PT = D // 128   # 4 partition tiles of the model dim
CT = S // 128   # 4 tiles of the sequence dim

_CACHE = {}


def _lhsT_layout(w):
    """[K, M] weight -> SBUF lhsT tile layout [128, (K//128)*M]."""
    k, m = w.shape
    c = k // 128
    return np.ascontiguousarray(
        w.reshape(c, 128, m).transpose(1, 0, 2).reshape(128, c * m)
    ).astype(np.float32)


def _build_program():
    import concourse.bass as bass
    import concourse.tile as tile
    from concourse import mybir, bacc
    from contextlib import ExitStack

    f32 = mybir.dt.float32
    bf16 = mybir.dt.bfloat16
    AF = mybir.ActivationFunctionType
    ALU = mybir.AluOpType

    nc = bacc.Bacc("TRN2", target_bir_lowering=False, debug=False)

    XT = nc.dram_tensor("xt", [128, 3, 512], bf16, kind="ExternalInput").ap()
    INW = nc.dram_tensor("inw", [128, 3 * 512], bf16, kind="ExternalInput").ap()
    INB = nc.dram_tensor("inb", [128, 4], f32, kind="ExternalInput").ap()
    WST = nc.dram_tensor("wst", [NL, 5, 128, 2048], bf16, kind="ExternalInput").ap()
    UST = nc.dram_tensor("ust", [NL, 1, 4 * 512], bf16, kind="ExternalInput").ap()
    BG = nc.dram_tensor("bg", [NL, 128, 4], f32, kind="ExternalInput").ap()
    BO = nc.dram_tensor("bo", [NL, 128, 4], f32, kind="ExternalInput").ap()
    W1T = nc.dram_tensor("w1t", [128, 4, 2048], bf16, kind="ExternalInput").ap()
    W2T = nc.dram_tensor("w2t", [128, 16 * 512], bf16, kind="ExternalInput").ap()
    B1C = nc.dram_tensor("b1c", [128, 16], f32, kind="ExternalInput").ap()
    B2C = nc.dram_tensor("b2c", [128, 4], f32, kind="ExternalInput").ap()
    ONWC = nc.dram_tensor("onwc", [128, 4], f32, kind="ExternalInput").ap()
    ONBC = nc.dram_tensor("onbc", [128, 4], f32, kind="ExternalInput").ap()
    CQ = nc.dram_tensor("cq", [1, 512], f32, kind="ExternalInput").ap()
    DK = nc.dram_tensor("dk", [1, 512], f32, kind="ExternalInput").ap()
    TRIU = nc.dram_tensor("triu", [128, 128], f32, kind="ExternalInput").ap()
    ONESC = nc.dram_tensor("onesc", [128, 1], bf16, kind="ExternalInput").ap()
    ONESR = nc.dram_tensor("onesr", [1, 128], bf16, kind="ExternalInput").ap()
    HOUT = nc.dram_tensor("hout", [4, 128, 512], bf16, kind="ExternalOutput").ap()

    with tile.TileContext(nc) as tc:
        with ExitStack() as ctx:
            consts = ctx.enter_context(tc.tile_pool(name="consts", bufs=1))
            wpool = ctx.enter_context(tc.tile_pool(name="wpool", bufs=10))
            wsmall = ctx.enter_context(tc.tile_pool(name="wsmall", bufs=2))
            hpool = ctx.enter_context(tc.tile_pool(name="hpool", bufs=2))
            apool = ctx.enter_context(tc.tile_pool(name="apool", bufs=1))
            atpool = ctx.enter_context(tc.tile_pool(name="atpool", bufs=16))
            spool = ctx.enter_context(tc.tile_pool(name="spool", bufs=6))
            sqpool = ctx.enter_context(tc.tile_pool(name="sqpool", bufs=4))
            grpool = ctx.enter_context(tc.tile_pool(name="grpool", bufs=4))
            retpool = ctx.enter_context(tc.tile_pool(name="retpool", bufs=1))
            plpool = ctx.enter_context(tc.tile_pool(name="plpool", bufs=3))
            odpool = ctx.enter_context(tc.tile_pool(name="odpool", bufs=2))
            f1pool = ctx.enter_context(tc.tile_pool(name="f1pool", bufs=3))
            psum = ctx.enter_context(tc.tile_pool(name="psum", bufs=5, space="PSUM"))
            bcps = ctx.enter_context(tc.tile_pool(name="bcps", bufs=2, space="PSUM"))
            rpsum = ctx.enter_context(tc.tile_pool(name="rpsum", bufs=1, space="PSUM"))

            # ---- consts ----
            cq_sb = consts.tile([1, 512], f32)
            nc.sync.dma_start(out=cq_sb[:], in_=CQ[:])
            dk_sb = consts.tile([1, 512], f32)
            nc.sync.dma_start(out=dk_sb[:], in_=DK[:])
            triu_sb = consts.tile([128, 128], f32)
            nc.sync.dma_start(out=triu_sb[:], in_=TRIU[:])
            onesc_sb = consts.tile([128, 1], bf16)
            nc.sync.dma_start(out=onesc_sb[:], in_=ONESC[:])
            onesr_sb = consts.tile([1, 128], bf16)
            nc.sync.dma_start(out=onesr_sb[:], in_=ONESR[:])
            onw_sb = consts.tile([128, 4], f32)
            nc.sync.dma_start(out=onw_sb[:], in_=ONWC[:])
            onb_sb = consts.tile([128, 4], f32)
            nc.sync.dma_start(out=onb_sb[:], in_=ONBC[:])
            b1_sb = consts.tile([128, 16], f32)
            nc.sync.dma_start(out=b1_sb[:], in_=B1C[:])
            b2_sb = consts.tile([128, 4], f32)
            nc.sync.dma_start(out=b2_sb[:], in_=B2C[:])
            eps_sb = consts.tile([128, 1], f32)
            nc.gpsimd.memset(eps_sb[:], EPS)
            # resident FFN weights (bf16: 2 MB each)
            w1_sb = consts.tile([128, 4, 2048], bf16)
            nc.sync.dma_start(out=w1_sb[:], in_=W1T[:])
            w2_sb = consts.tile([128, 16 * 512], bf16)
            nc.sync.dma_start(out=w2_sb[:], in_=W2T[:])

            # ---- PE warm-up burst ----
            # The PE is idle for ~10us at kernel start while input/weight
            # DMAs stream. ~5us of back-to-back dummy matmuls here crosses
            # the HAM activity window, so the clock gate opens to 8/8
            # before the first real matmul regardless of the throttle
            # state the device was left in (this start-state is the likely
            # cause of occasional ~20% slower whole-run timings).
            wa = consts.tile([128, 128], bf16)
            nc.gpsimd.memset(wa[:], 0.0)
            wm = consts.tile([128, 512], bf16)
            nc.gpsimd.memset(wm[:], 0.0)
            wp = psum.tile([128, 512], f32, tag="big")
            for _ in range(24):
                nc.tensor.matmul(wp[:], wa[:], wm[:], start=True, stop=True)

            # ---- input projection: ht = (x @ in_w + in_b)^T ----
            inctx = ExitStack()
            inpool = inctx.enter_context(tc.tile_pool(name="inpool", bufs=1))
            xt_sb = inpool.tile([128, 3, 512], bf16)
            nc.sync.dma_start(out=xt_sb[:], in_=XT[:])
            inw_sb = inpool.tile([128, 3 * 512], bf16)
            nc.sync.dma_start(out=inw_sb[:], in_=INW[:])
            inb_sb = inpool.tile([128, 4], f32)
            nc.sync.dma_start(out=inb_sb[:], in_=INB[:])
            ht = []
            for j in range(PT):
                p = psum.tile([128, 512], f32, tag="big")
                for c in range(3):
                    nc.tensor.matmul(
                        p[:], inw_sb[:, c * 512 + 128 * j : c * 512 + 128 * (j + 1)],
                        xt_sb[:, c, :], start=(c == 0), stop=(c == 2))
                hj = hpool.tile([128, 512], bf16, tag=f"ht{j}")
                nc.scalar.activation(hj[:], p[:], AF.Identity, bias=inb_sb[:, j : j + 1])
                ht.append(hj)
            inctx.close()

            def load_weights(lidx):
                wmat = []
                for i in range(5):
                    wt = wpool.tile([128, 2048], bf16, tag="wmat")
                    nc.sync.dma_start(out=wt[:], in_=WST[lidx, i])
                    wmat.append(wt)
                u_sb = wsmall.tile([1, 4 * 512], bf16, tag="urow")
                nc.sync.dma_start(out=u_sb[:], in_=UST[lidx])
                bg_sb = wsmall.tile([128, 4], f32, tag="bgc")
                nc.sync.dma_start(out=bg_sb[:], in_=BG[lidx])
                bo_sb = wsmall.tile([128, 4], f32, tag="boc")
                nc.sync.dma_start(out=bo_sb[:], in_=BO[lidx])
                return wmat, u_sb, bg_sb, bo_sb

            def layer_stats(tiles, sq_tiles):
                """ones-matmul stats -> (sums_ps, ssq_ps) [1,512] PSUM tiles."""
                sums = bcps.tile([1, 512], f32, tag="bc")
                ssq = bcps.tile([1, 512], f32, tag="bc")
                for j in range(PT):
                    nc.tensor.matmul(sums[:], onesc_sb[:], tiles[j][:],
                                     start=(j == 0), stop=(j == PT - 1))
                for j in range(PT):
                    nc.tensor.matmul(ssq[:], onesc_sb[:], sq_tiles[j][:],
                                     start=(j == 0), stop=(j == PT - 1))
                return sums, ssq

            def rsqrt_row(vrow_ps, ssq_ps, pool):
                """sums/ssq psum rows -> (negmu bf16, rb bf16 [1,512]).

                var via 3 DVE ops, sqrt(var+eps) on ACT (~570ns, short
                serial latency), 1/std via the single-instruction DVE
                reciprocal_approx_fast."""
                negmu = pool.tile([1, 512], bf16, tag="tiny")
                nc.vector.tensor_scalar_mul(negmu[:], vrow_ps[:], -1.0 / D)
                nm2 = pool.tile([1, 512], f32, tag="tiny")
                nc.vector.tensor_mul(nm2[:], negmu[:], negmu[:])
                w = pool.tile([1, 512], f32, tag="tiny")
                nc.vector.scalar_tensor_tensor(w[:], ssq_ps[:], 1.0 / D, nm2[:],
                                               ALU.mult, ALU.subtract)
                std = pool.tile([1, 512], f32, tag="tiny")
                nc.scalar.activation(std[:], w[:], AF.Sqrt, bias=eps_sb[0:1, :])
                r = pool.tile([1, 512], f32, tag="tiny")
                nc.vector.reciprocal_approx_fast(r[:], std[:])
                rb = pool.tile([1, 512], bf16, tag="tiny")
                nc.vector.tensor_scalar_mul(rb[:], r[:], 1.0)
                return negmu, r, rb

            def bcast_plane(row_bf16):
                """[1,512] bf16 row -> [128,512] bf16 SBUF plane."""
                p = bcps.tile([128, 512], f32, tag="bc")
                nc.tensor.matmul(p[:], onesr_sb[:], row_bf16[:], start=True, stop=True)
                sb = plpool.tile([128, 512], bf16, tag="plane")
                nc.scalar.copy(sb[:], p[:])
                return sb

            def retention(lidx, weights, next_weights_hook=None):
                wmat, u_sb, bg_sb, bo_sb = weights

                # pre-LN stats
                sq = []
                for j in range(PT):
                    s = sqpool.tile([128, 512], bf16, tag="sq")
                    nc.scalar.activation(s[:], ht[j][:], AF.Square)
                    sq.append(s)
                sums, ssq = layer_stats(ht, sq)
                negmu, r, rb = rsqrt_row(sums, ssq, spool)
                qs = spool.tile([1, 512], bf16, tag="tiny")
                nc.vector.tensor_mul(qs[:], r[:], cq_sb[:])
                r2 = spool.tile([1, 512], f32, tag="tiny")
                nc.vector.tensor_mul(r2[:], r[:], r[:])
                ks = spool.tile([1, 512], bf16, tag="tiny")
                nc.vector.tensor_mul(ks[:], r2[:], dk_sb[:])
                qs_b = ks_b = g_b = None

                # prefetch next layer's weights while this layer computes
                nxt = next_weights_hook() if next_weights_hook else None

                # projections, V first: its drain is a plain copy with no
                # rsqrt dependency, giving the LN chain V+Q matmul runway
                qt, kt, vn, gt = [], [], [], []
                for ip, dest in ((2, vn), (0, qt), (1, kt), (3, gt)):
                    pss = []
                    for j in range(PT):
                        p = psum.tile([128, 512], f32, tag="big")
                        if ip == 2:
                            for c in range(PT):
                                nc.tensor.matmul(
                                    p[:], ht[c][:, 128 * j : 128 * (j + 1)],
                                    wmat[2][:, c * 512 : (c + 1) * 512],
                                    start=(c == 0), stop=False)
                        else:
                            for c in range(PT):
                                nc.tensor.matmul(
                                    p[:],
                                    wmat[ip][:, c * 512 + 128 * j : c * 512 + 128 * (j + 1)],
                                    ht[c][:], start=(c == 0), stop=False)
                        pss.append(p)
                    for j in range(PT):
                        p = pss[j]
                        if ip == 2:
                            nc.tensor.matmul(
                                p[:], negmu[:, 128 * j : 128 * (j + 1)],
                                u_sb[:, ip * 512 : ip * 512 + 512],
                                start=False, stop=True)
                        else:
                            nc.tensor.matmul(
                                p[:], u_sb[:, ip * 512 + 128 * j : ip * 512 + 128 * (j + 1)],
                                negmu[:], start=False, stop=True)
                    if ip == 0:
                        qs_b = bcast_plane(qs)
                        ks_b = bcast_plane(ks)
                        g_b = bcast_plane(rb)
                    for j in range(PT):
                        p = pss[j]
                        t = apool.tile([128, 512], bf16, tag=f"proj{ip}_{j}")
                        if ip == 0:
                            nc.vector.tensor_mul(t[:], p[:], qs_b[:])
                        elif ip == 1:
                            nc.vector.tensor_mul(t[:], p[:], ks_b[:])
                        elif ip == 2:
                            nc.scalar.copy(t[:], p[:])
                        else:
                            tg = odpool.tile([128, 512], bf16, tag="gtmp")
                            nc.vector.tensor_mul(tg[:], p[:], g_b[:])
                            nc.scalar.activation(t[:], tg[:], AF.Sigmoid,
                                                 bias=bg_sb[:, j : j + 1])
                        dest.append(t)

                # scores + AV per head pair (heads 2jt, 2jt+1 share q/k tile
                # jt); inner-LN stats accumulate per jt as ret tiles land, so
                # only the rsqrt chain remains after the last AV drain
                ret_sb = []
                sums2 = bcps.tile([1, 512], f32, tag="bc")
                ssq2 = bcps.tile([1, 512], f32, tag="bc")

                def score_block(jt):
                    # score matmuls interleaved per k_t: head A's lhsT at
                    # rows 0-63, head B's at 64-127 -> concurrent (row tiling)
                    at_pair = [[], []]
                    for k_t in range(CT):
                        qstart = 128 * k_t
                        npr = 512 - qstart
                        scp = []
                        for i, r0 in enumerate((0, 64)):
                            sc = psum.tile([128, 512], f32, tag="big")
                            nc.tensor.matmul(
                                sc[:, 0:npr],
                                kt[jt][r0 : r0 + 64, 128 * k_t : 128 * (k_t + 1)],
                                qt[jt][r0 : r0 + 64, qstart : 512],
                                start=True, stop=True)
                            scp.append(sc)
                        for i in range(2):
                            sc = scp[i]
                            at = atpool.tile([128, 512], bf16, tag="at")
                            # diagonal 128-block: triangular mask (DVE)
                            nc.vector.tensor_mul(
                                at[:, qstart : qstart + 128],
                                sc[:, 0:128], triu_sb[:])
                            # off-diagonal region: plain drain (ACT)
                            if npr > 128:
                                nc.scalar.copy(
                                    at[:, qstart + 128 : 512], sc[:, 128:npr])
                            at_pair[i].append(at)
                    return at_pair

                # pipeline skew of one head pair: the score matmuls for
                # jt+1 are issued BEFORE the AV matmuls for jt, so the
                # in-order PE runs them while jt's at drains complete
                # instead of stalling in front of AV. The inner-LN stats
                # matmuls are deferred one iteration for the same reason:
                # they wait on the ACT ret-copy+Square chain and would
                # otherwise gate the next score block in the PE queue.
                def emit_stats(pend):
                    prs, ps2, pjt = pend
                    nc.tensor.matmul(sums2[:], onesc_sb[:], prs[:],
                                     start=(pjt == 0), stop=(pjt == PT - 1))
                    nc.tensor.matmul(ssq2[:], onesc_sb[:], ps2[:],
                                     start=(pjt == 0), stop=(pjt == PT - 1))

                cur = score_block(0)
                pend = None
                for jt in range(PT):
                    nxt_at = score_block(jt + 1) if jt + 1 < PT else None
                    if pend is not None:
                        emit_stats(pend)
                    at_pair = cur
                    # AV pair interleaved per k_t: head A -> PSUM cols 0-63,
                    # head B -> 64-127 (col tiling, concurrent)
                    rp = rpsum.tile([128, 512], f32, tag="ret")
                    for k_t in range(CT):
                        qstart = 128 * k_t
                        for i in range(2):
                            h = 2 * jt + i
                            nc.tensor.matmul(
                                rp[64 * i : 64 * i + 64, qstart : 512],
                                vn[k_t][:, 64 * h : 64 * (h + 1)],
                                at_pair[i][k_t][:, qstart : 512],
                                start=(k_t == 0), stop=(k_t == CT - 1),
                                tile_position=(0, 64 * i))
                    rs = retpool.tile([128, 512], bf16, tag=f"ret{jt}")
                    nc.scalar.copy(rs[:], rp[:])
                    ret_sb.append(rs)
                    s2 = sqpool.tile([128, 512], bf16, tag="sq")
                    nc.scalar.activation(s2[:], rs[:], AF.Square)
                    pend = (rs, s2, jt)
                    cur = nxt_at
                emit_stats(pend)

                negmu2, rB, rBb = rsqrt_row(sums2, ssq2, spool)
                nm2_b = bcast_plane(negmu2)

                # gret = (ret - mu2) * g; rstd2 is applied at the O drain so
                # the O matmuls only wait on negmu2, not the whole chain
                gret = []
                for j in range(PT):
                    tmpc = odpool.tile([128, 512], bf16, tag="odb")
                    nc.vector.tensor_add(tmpc[:], ret_sb[j][:], nm2_b[:])
                    gr = grpool.tile([128, 512], bf16, tag="gret")
                    nc.vector.tensor_mul(gr[:], tmpc[:], gt[j][:])
                    gret.append(gr)

                # O = rstd2*T1 + bo + h, matmuls c-outer to start on gret[0]
                p1s = []
                for j in range(PT):
                    p1 = psum.tile([128, 512], f32, tag="big")
                    p1s.append(p1)
                for c in range(PT):
                    for j in range(PT):
                        nc.tensor.matmul(
                            p1s[j][:],
                            wmat[4][:, c * 512 + 128 * j : c * 512 + 128 * (j + 1)],
                            gret[c][:], start=(c == 0), stop=(c == PT - 1))
                    if c == 0:
                        rstd2_b = bcast_plane(rBb)
                newht = []
                for j in range(PT):
                    a = odpool.tile([128, 512], bf16, tag="oda")
                    nc.vector.tensor_mul(a[:], p1s[j][:], rstd2_b[:])
                    hn = hpool.tile([128, 512], bf16, tag=f"ht{j}")
                    nc.vector.scalar_tensor_tensor(hn[:], a[:], bo_sb[:, j : j + 1],
                                                   ht[j][:], ALU.add, ALU.add)
                    newht.append(hn)
                for j in range(PT):
                    ht[j] = newht[j]
                return nxt

            def ffn():
                f2ps = []
                for _j in range(PT):
                    f2p = psum.tile([128, 512], f32, tag="big")
                    f2ps.append(f2p)

                def w1_group(t):
                    # W1 PSUM tiles come from the bcps pool (idle during the
                    # FFN) so two can be in flight alongside the 4 f2ps banks
                    p = bcps.tile([128, 512], f32, tag="bc")
                    for c in range(PT):
                        nc.tensor.matmul(
                            p[:], w1_sb[:, c, 128 * t : 128 * (t + 1)], ht[c][:],
                            start=(c == 0), stop=(c == PT - 1))
                    return p

                # software-pipeline skew of one tile: W1 for t+1 is issued
                # BEFORE W2 for t, so the in-order PE never stalls on the
                # gelu drain between a W1 group and its W2 group
                pW = w1_group(0)
                for t in range(16):
                    pN = w1_group(t + 1) if t + 1 < 16 else None
                    f1 = f1pool.tile([128, 512], bf16, tag="f1")
                    nc.scalar.activation(f1[:], pW[:], AF.Gelu, bias=b1_sb[:, t : t + 1])
                    for j in range(PT):
                        nc.tensor.matmul(
                            f2ps[j][:], w2_sb[:, t * 512 + 128 * j : t * 512 + 128 * (j + 1)],
                            f1[:], start=(t == 0), stop=(t == 15))
                    pW = pN
                for j in range(PT):
                    hn = hpool.tile([128, 512], bf16, tag=f"ht{j}")
                    nc.vector.scalar_tensor_tensor(hn[:], f2ps[j][:], b2_sb[:, j : j + 1],
                                                   ht[j][:], ALU.add, ALU.add)
                    ht[j] = hn

            # layer schedule with one-layer weight prefetch
            weights = load_weights(0)
            for l in range(N_WM):
                hook = (lambda nl=l + 1: load_weights(nl)) if l + 1 < NL else None
                weights = retention(l, weights, hook) or weights
                ffn()

            # final LN of world model
            sqf = []
            for j in range(PT):
                s = sqpool.tile([128, 512], bf16, tag="sq")
                nc.scalar.activation(s[:], ht[j][:], AF.Square)
                sqf.append(s)
            sumsf, ssqf = layer_stats(ht, sqf)
            negmuf, rf, rfb = rsqrt_row(sumsf, ssqf, spool)
            nmr = spool.tile([1, 512], bf16, tag="tiny")
            nc.vector.tensor_mul(nmr[:], negmuf[:], rf[:])
            rf_b = bcast_plane(rfb)
            nmr_b = bcast_plane(nmr)
            for j in range(PT):
                t1 = odpool.tile([128, 512], bf16, tag="oda")
                nc.vector.tensor_mul(t1[:], ht[j][:], rf_b[:])
                t2 = odpool.tile([128, 512], bf16, tag="odb")
                nc.vector.tensor_add(t2[:], t1[:], nmr_b[:])
                hn = hpool.tile([128, 512], bf16, tag=f"ht{j}")
                nc.vector.tensor_scalar(hn[:], t2[:], onw_sb[:, j : j + 1],
                                        onb_sb[:, j : j + 1], ALU.mult, ALU.add)
                ht[j] = hn

            # retention core layers
            for l in range(N_WM, NL):
                hook = (lambda nl=l + 1: load_weights(nl)) if l + 1 < NL else None
                weights = retention(l, weights, hook) or weights

            for j in range(PT):
                nc.sync.dma_start(out=HOUT[j], in_=ht[j][:])

    nc.compile()
    return nc


def _host_prep(inputs):
    """Fold weights host-side; returns the shared in_map dict (no xt)."""
    g = {k: np.asarray(v, dtype=np.float32) for k, v in inputs.items()}

    def layer_params(l):
        if l < N_WM:
            pre = "wm_"
            i = l
        else:
            pre = "co_"
            i = l - N_WM
        return {n: g[pre + n][i] for n in
                ("wq", "bq", "wk", "bk", "wv", "bv", "wg", "bg", "wo", "bo",
                 "lnw", "lnb", "prew", "preb")}

    wst = np.zeros((NL, 5, 128, 2048), np.float32)
    ust = np.zeros((NL, 1, 4 * 512), np.float32)
    bgc = np.zeros((NL, 128, 4), np.float32)
    boc = np.zeros((NL, 128, 4), np.float32)
    for l in range(NL):
        p = layer_params(l)
        wq = p["prew"][:, None] * p["wq"]
        wk = p["prew"][:, None] * p["wk"]
        wv = p["prew"][:, None] * p["wv"]
        wg = p["prew"][:, None] * p["wg"]
        wo = p["lnw"][:, None] * p["wo"]
        # biases bq~ = bq + preb @ wq must be zero for this folded fast path
        for nm, w in (("bq", p["wq"]), ("bk", p["wk"]), ("bv", p["wv"])):
            bb = p[nm] + p["preb"] @ w
            assert np.abs(bb).max() == 0.0, f"nonzero {nm} not supported"
        assert np.abs(p["lnb"]).max() == 0.0, "nonzero lnb not supported"
        bgf = p["bg"] + p["preb"] @ p["wg"]
        wst[l, 0] = _lhsT_layout(wq)
        wst[l, 1] = _lhsT_layout(wk)
        wst[l, 2] = _lhsT_layout(wv)
        wst[l, 3] = _lhsT_layout(wg)
        wst[l, 4] = _lhsT_layout(wo)
        ust[l, 0, 0:512] = wq.sum(0)
        ust[l, 0, 512:1024] = wk.sum(0)
        ust[l, 0, 1024:1536] = wv.sum(0)
        ust[l, 0, 1536:2048] = wg.sum(0)
        bgc[l] = bgf.reshape(4, 128).T
        boc[l] = p["bo"].reshape(4, 128).T

    inw = _lhsT_layout(g["in_w"])
    inb = g["in_b"].reshape(4, 128).T.copy()
    w1t = _lhsT_layout(g["ffn_w1"]).reshape(128, 4, 2048)
    w2t = _lhsT_layout(g["ffn_w2"])  # [128, 16*512]
    b1c = g["ffn_b1"].reshape(16, 128).T.copy()
    b2c = g["ffn_b2"].reshape(4, 128).T.copy()
    onwc = g["wm_onw"].reshape(4, 128).T.copy()
    onbc = g["wm_onb"].reshape(4, 128).T.copy()

    q = np.arange(S, dtype=np.float64)
    cq = (DECAY ** q / np.sqrt(DH)).astype(np.float32).reshape(1, 512)
    dk = (DECAY ** (-q)).astype(np.float32).reshape(1, 512)
    triu = np.triu(np.ones((128, 128), np.float32))

    return {
        "inw": inw.astype(BF16), "inb": inb,
        "wst": wst.astype(BF16), "ust": ust.astype(BF16),
        "bg": bgc, "bo": boc,
        "w1t": np.ascontiguousarray(w1t).astype(BF16),
        "w2t": w2t.astype(BF16), "b1c": b1c, "b2c": b2c,
        "onwc": onwc, "onbc": onbc, "cq": cq, "dk": dk, "triu": triu,
        "onesc": np.ones((128, 1), BF16),
        "onesr": np.ones((1, 128), BF16),
    }


def _make_in_maps(inputs):
    shared = _host_prep(inputs)
    x = np.asarray(inputs["x"], dtype=np.float32)
    in_maps = []
    for b in range(B):
        xt = np.ascontiguousarray(
            x[b].T.reshape(3, 128, 512).transpose(1, 0, 2)).astype(BF16)
        m = dict(shared)
        m["xt"] = xt
        in_maps.append(m)
    return in_maps


def kernel(**inputs):
    from concourse.bass_utils import run_bass_kernel_spmd

    if "nc" not in _CACHE:
        _CACHE["nc"] = _build_program()
    nc = _CACHE["nc"]

    in_maps = _make_in_maps(inputs)
    res = run_bass_kernel_spmd(nc, in_maps, list(range(B)))
    out = np.empty((B, S, D), np.float32)
    for b in range(B):
        hout = res.results[b]["hout"]  # [4,128,512] = ht tiles (transposed h)
        out[b] = np.asarray(hout, dtype=np.float32).reshape(512, 512).T
    return out
